# revision 1
# baseline (speedup 1.0000x reference)
"""Redesigned Trainium2 Bass kernel for tf-idf embedding pooling + MLP.

Math identity: pooled[b] = sum_t c_{b,t}^2 * idf_t * emb_t where c = per-row
token counts. Per-core histogram of its 8 batch rows via one-hot matmuls
(tok = hi*256 + lo; H[lo,hi] += onehot_lo^T @ onehot_hi), then a = H^2*idf,
AllToAll redistributes a by vocab shard, each core contracts its 6400-row
bf16 emb shard for all 64 rows, ReduceScatter returns final pooled rows.

Changes vs the old baseline:
- all matmuls bf16 (fp32 matmul is 4x slower on the PE)
- emb shard converted to bf16 on host: 3.3MB HBM read per core (was 51.2MB)
- hi one-hot on DVE/Pool via is_equal (was 2 ACT ops: ~90us of ACT time)
- a2a payload bf16 with 400B-run layout both sides
- single DMA for tokens (host pre-layout)
"""

import os
import sys

import numpy as np

sys.path.insert(0, "/opt/trn_rl_repo")

import concourse.bass as bass  # noqa: E402,F401
import concourse.mybir as mybir  # noqa: E402
import concourse.tile as tile  # noqa: E402
from concourse import bacc  # noqa: E402
from concourse.masks import make_identity  # noqa: E402

P = 128
S = 2048
B = 64
D = 256
V = 50000
NCORES = 8
RPC = B // NCORES  # 8 rows per core
NLO = 256
NHI = 200  # 196 real hi values + 4 pad (tokens never land there)
NHL = NHI // NCORES  # 25 hi rows per vocab shard
VSH = NHL * NLO  # 6400 vocab rows per shard
VPAD = NHI * NLO  # 51200
STILES = S // P  # 16
NT = RPC * STILES  # 128 token tiles of 128

F32 = mybir.dt.float32
BF16 = mybir.dt.bfloat16
I32 = mybir.dt.int32

# which engine computes the hi one-hot for token tile index t (0..127):
# balance point from TimelineSim: Pool rate ~373ns/tile vs DVE ~111ns —
# even split keeps both engines at ~26-28us for the histogram phase
HI_ON_POOL = lambda t: (t % 2) == 0  # noqa: E731

_CACHE = {}


def _mlp_tail(nc, tc, cpool, ps_mlp, pooled_sb, identity,
              w1t_sb, b1_sb, w2t_sb, b2a_sb, b2b_sb, w3a_sb, w3b_sb, b3_sb, out):
    """pooled_sb [RPC, 256] f32 -> softmax out DMA. (same as old baseline)"""
    pooledT = cpool.tile([P, 2, RPC], F32, tag="pooledT", bufs=2)
    for kc in range(2):
        ptp = ps_mlp.tile([P, RPC], F32, tag="ptp")
        nc.tensor.transpose(
            ptp[:, :], pooled_sb[:, kc * P : (kc + 1) * P], identity[:RPC, :RPC]
        )
        nc.vector.tensor_copy(pooledT[:, kc, :], ptp[:, :])

    h1_ps = ps_mlp.tile([100, RPC], F32, tag="h1")
    for kc in range(2):
        nc.tensor.matmul(
            h1_ps[:, :], lhsT=w1t_sb[:, kc, :], rhs=pooledT[:, kc, :],
            start=(kc == 0), stop=(kc == 1),
        )
    h1_sb = cpool.tile([100, RPC], F32, tag="h1_sb", bufs=2)
    nc.scalar.activation(
        h1_sb[:], h1_ps[:, :], mybir.ActivationFunctionType.Relu,
        bias=b1_sb[:, 0:1], scale=1.0,
    )

    h2a_ps = ps_mlp.tile([P, RPC], F32, tag="h2a")
    nc.tensor.matmul(h2a_ps[:, :], lhsT=w2t_sb[:, 0:128], rhs=h1_sb[:, :],
                     start=True, stop=True)
    h2b_ps = ps_mlp.tile([22, RPC], F32, tag="h2b")
    nc.tensor.matmul(h2b_ps[:, :], lhsT=w2t_sb[:, 128:150], rhs=h1_sb[:, :],
                     start=True, stop=True)
    h2a_sb = cpool.tile([P, RPC], F32, tag="h2a_sb", bufs=2)
    h2b_sb = cpool.tile([22, RPC], F32, tag="h2b_sb", bufs=2)
    nc.scalar.activation(h2a_sb[:], h2a_ps[:, :],
                         mybir.ActivationFunctionType.Relu,
                         bias=b2a_sb[:, 0:1], scale=1.0)
    nc.scalar.activation(h2b_sb[:], h2b_ps[:, :],
                         mybir.ActivationFunctionType.Relu,
                         bias=b2b_sb[:, 0:1], scale=1.0)

    lg_ps = ps_mlp.tile([2, RPC], F32, tag="lg")
    nc.tensor.matmul(lg_ps[:, :], lhsT=w3a_sb[:, :], rhs=h2a_sb[:, :],
                     start=True, stop=False)
    nc.tensor.matmul(lg_ps[:, :], lhsT=w3b_sb[:, :], rhs=h2b_sb[:, :],
                     start=False, stop=True)
    lg_sb = cpool.tile([2, RPC], F32, tag="lg_sb", bufs=2)
    nc.scalar.add(lg_sb[:], lg_ps[:, :], b3_sb[:, 0:1])

    lt_ps = ps_mlp.tile([RPC, 2], F32, tag="lt")
    nc.tensor.transpose(lt_ps[:, :], lg_sb[:, :], identity[:2, :2])
    e_sb = cpool.tile([RPC, 2], F32, tag="e_sb", bufs=2)
    nc.scalar.activation(e_sb[:], lt_ps[:, :], mybir.ActivationFunctionType.Exp)
    ssum = cpool.tile([RPC, 1], F32, tag="ssum", bufs=2)
    nc.vector.tensor_reduce(ssum[:], e_sb[:], axis=mybir.AxisListType.X,
                            op=mybir.AluOpType.add)
    rinv = cpool.tile([RPC, 1], F32, tag="rinv", bufs=2)
    nc.vector.reciprocal(rinv[:], ssum[:])
    res_sb = cpool.tile([RPC, 2], F32, tag="res_sb", bufs=2)
    nc.vector.tensor_scalar(out=res_sb[:], in0=e_sb[:], scalar1=rinv[:, 0:1],
                            scalar2=None, op0=mybir.AluOpType.mult)
    nc.sync.dma_start(out[:, :], res_sb[:])


def _build_nc(reps=1, lhsT_3d=True):
    nc = bacc.Bacc(None, target_bir_lowering=False, debug=False)

    xt = nc.dram_tensor("xt", [P, NT], I32, kind="ExternalInput")
    embs = nc.dram_tensor("embs", [VSH, D], BF16, kind="ExternalInput")
    idf_t = nc.dram_tensor("idf_t", [NLO, NHI], BF16, kind="ExternalInput")
    w1t = nc.dram_tensor("w1t", [256, 100], F32, kind="ExternalInput")
    b1 = nc.dram_tensor("b1", [100], F32, kind="ExternalInput")
    w2t = nc.dram_tensor("w2t", [100, 150], F32, kind="ExternalInput")
    b2 = nc.dram_tensor("b2", [150], F32, kind="ExternalInput")
    w3t = nc.dram_tensor("w3t", [150, 2], F32, kind="ExternalInput")
    b3 = nc.dram_tensor("b3", [2], F32, kind="ExternalInput")
    out = nc.dram_tensor("out", [RPC, 2], F32, kind="ExternalOutput")

    with tile.TileContext(nc) as tc:
        with (
            tc.tile_pool(name="const", bufs=1) as cpool,
            tc.tile_pool(name="emb", bufs=2) as epool,
            tc.tile_pool(name="work", bufs=2) as wpool,
            tc.tile_pool(name="oh", bufs=8) as ohpool,
            tc.tile_pool(name="sq", bufs=4) as sqpool,
            tc.tile_pool(name="dram", bufs=2, space="DRAM") as dpool,
            tc.tile_pool(name="ps_acc", bufs=2, space="PSUM") as ps_acc,
        ):
            # ---------- constants ----------
            iota_i32 = cpool.tile([P, NLO], I32)
            nc.gpsimd.iota(iota_i32[:], pattern=[[1, NLO]], base=0,
                           channel_multiplier=0)
            iota_lo = cpool.tile([P, NLO], BF16)
            nc.vector.tensor_copy(iota_lo[:], iota_i32[:])
            iota_hi = cpool.tile([P, NHI], BF16)
            nc.vector.tensor_copy(iota_hi[:], iota_i32[:, :NHI])

            identity = cpool.tile([P, P], F32)
            make_identity(nc, identity[:])

            idf_sb = cpool.tile([P, 2, NHI], BF16)
            nc.sync.dma_start(idf_sb[:, 0, :], idf_t[0:128, :])
            nc.sync.dma_start(idf_sb[:, 1, :], idf_t[128:256, :])

            w1t_sb = cpool.tile([P, 2, 100], F32)
            nc.sync.dma_start(w1t_sb[:, :, :],
                              w1t[:, :].rearrange("(c p) m -> p c m", p=P))
            b1_sb = cpool.tile([100, 1], F32)
            nc.sync.dma_start(b1_sb[:, :], b1[:, None])
            w2t_sb = cpool.tile([100, 150], F32)
            nc.sync.dma_start(w2t_sb[:, :], w2t[:, :])
            b2a_sb = cpool.tile([128, 1], F32)
            b2b_sb = cpool.tile([22, 1], F32)
            nc.sync.dma_start(b2a_sb[:, :], b2[:128, None])
            nc.sync.dma_start(b2b_sb[:, :], b2[128:150, None])
            w3a_sb = cpool.tile([128, 2], F32)
            w3b_sb = cpool.tile([22, 2], F32)
            nc.sync.dma_start(w3a_sb[:, :], w3t[0:128, :])
            nc.sync.dma_start(w3b_sb[:, :], w3t[128:150, :])
            b3_sb = cpool.tile([2, 1], F32)
            nc.sync.dma_start(b3_sb[:, :], b3[:, None])

            for _rep in range(reps):
                # ---- tokens FIRST: the tiny tok DMA must not queue behind
                # the 3.3MB emb DMA (FIFO per queue stalls the histogram) ----
                tok_i32 = wpool.tile([P, NT], I32, tag="tok")
                nc.sync.dma_start(tok_i32[:, :], xt[:, :])

                # ---- emb shard preload (overlaps the whole histogram) ----
                emb_sb = epool.tile([P, VSH // P, D], BF16, tag="emb_sb")
                nc.sync.dma_start(
                    emb_sb[:, :, :],
                    embs[:, :].rearrange("(c p) d -> p c d", p=P),
                )
                lo_i32 = wpool.tile([P, NT], I32, tag="lo_i32")
                hi_i32 = wpool.tile([P, NT], I32, tag="hi_i32")
                nc.vector.tensor_scalar(
                    out=lo_i32[:], in0=tok_i32[:], scalar1=255, scalar2=None,
                    op0=mybir.AluOpType.bitwise_and)
                nc.vector.tensor_scalar(
                    out=hi_i32[:], in0=tok_i32[:], scalar1=8, scalar2=None,
                    op0=mybir.AluOpType.logical_shift_right)
                lo_f = wpool.tile([P, NT], F32, tag="lo_f")
                hi_f = wpool.tile([P, NT], F32, tag="hi_f")
                nc.vector.tensor_copy(lo_f[:], lo_i32[:])
                nc.vector.tensor_copy(hi_f[:], hi_i32[:])

                # ---- per-row histograms -> a = H^2 * idf (bf16) ----
                # a_sb layout [p, mh, hi, r]: per (dst_shard, mh) the
                # (hi-within-shard, r) block is 200 contiguous elements.
                a_sb = wpool.tile([P, 2, NHI, RPC], BF16, tag="a_sb")

                with tc.tile_pool(name="ps_ht", bufs=2, space="PSUM") as ps_ht:
                    for r in range(RPC):
                        ht_ps = [ps_ht.tile([P, NHI], F32, name=f"ht{mh}",
                                            tag=f"ht{mh}")
                                 for mh in range(2)]
                        for f in range(STILES):
                            t = r * STILES + f
                            lo_oh = ohpool.tile([P, NLO], BF16, tag="lo_oh")
                            hi_oh = ohpool.tile([P, NHI], BF16, tag="hi_oh")
                            nc.vector.tensor_scalar(
                                out=lo_oh[:], in0=iota_lo[:],
                                scalar1=lo_f[:, t : t + 1], scalar2=None,
                                op0=mybir.AluOpType.is_equal)
                            hi_eng = nc.gpsimd if HI_ON_POOL(t) else nc.vector
                            hi_eng.tensor_scalar(
                                out=hi_oh[:], in0=iota_hi[:],
                                scalar1=hi_f[:, t : t + 1], scalar2=None,
                                op0=mybir.AluOpType.is_equal)
                            for mh in range(2):
                                nc.tensor.matmul(
                                    ht_ps[mh][:, :],
                                    lhsT=lo_oh[:, mh * P : (mh + 1) * P],
                                    rhs=hi_oh[:, :],
                                    start=(f == 0), stop=(f == STILES - 1))
                        for mh in range(2):
                            sq = sqpool.tile([P, NHI], BF16, tag="sq")
                            nc.scalar.square(sq[:], ht_ps[mh][:, :])
                            tt_eng = nc.vector if (r + mh) % 2 else nc.gpsimd
                            tt_eng.tensor_tensor(
                                out=a_sb[:, mh, :, r], in0=sq[:],
                                in1=idf_sb[:, mh, :],
                                op=mybir.AluOpType.mult)

                # ---- AllToAll: a2a[dst][mh][p][hl][r] bf16 ----
                a2a_in = dpool.tile([NCORES, 2, P, NHL, RPC], BF16,
                                    tag="a2a_in")
                a2a_out = dpool.tile([NCORES, 2, P, NHL, RPC], BF16,
                                     tag="a2a_out")
                for mh in range(2):
                    nc.sync.dma_start(
                        a2a_in[:, mh, :, :, :].rearrange(
                            "dst p hl r -> p dst (hl r)"),
                        a_sb[:, mh, :, :].rearrange(
                            "p (dst hl) r -> p dst (hl r)", dst=NCORES),
                    )
                nc.gpsimd.collective_compute(
                    "AllToAll", mybir.AluOpType.bypass,
                    replica_groups=[list(range(NCORES))],
                    ins=[a2a_in[:, :, :, :, :]],
                    outs=[a2a_out[:, :, :, :, :]],
                )
                # receive into [p][src][mh][hl][r]
                recv_sb = wpool.tile([P, NCORES, 2, NHL, RPC], BF16,
                                     tag="recv_sb")
                for mh in range(2):
                    nc.sync.dma_start(
                        recv_sb[:, :, mh, :, :].rearrange(
                            "p src hl r -> p src (hl r)"),
                        a2a_out[:, mh, :, :, :].rearrange(
                            "src p hl r -> p src (hl r)"),
                    )
                # matmul weights need ONE free dim: reshuffle to
                # [p][mh][hl][(src r)] on the mostly-idle ACT engine
                recv_mm = wpool.tile([P, 2, NHL, NCORES, RPC], BF16,
                                     tag="recv_mm")
                for mh in range(2):
                    nc.scalar.activation(
                        recv_mm[:, mh, :, :, :],
                        recv_sb[:, :, mh, :, :].rearrange(
                            "p src hl r -> p hl src r"),
                        mybir.ActivationFunctionType.Copy, scale=1.0)

                # ---- pooled[64, 256] = sum over my vocab shard ----
                pooled_ps = ps_acc.tile([B, D], F32, tag="pooled")
                for c in range(VSH // P):  # 50 chunks of 128 vocab rows
                    hl, mh = c >> 1, c & 1
                    nc.tensor.matmul(
                        pooled_ps[:, :],
                        lhsT=recv_mm[:, mh, hl, :, :].rearrange(
                            "p src r -> p (src r)"),
                        rhs=emb_sb[:, c, :],
                        start=(c == 0), stop=(c == VSH // P - 1))
                pooled_full = wpool.tile([B, D], F32, tag="pooled_full")
                nc.vector.tensor_copy(pooled_full[:], pooled_ps[:, :])

                # ---- ReduceScatter -> my 8 rows ----
                rs_in = dpool.tile([B, D], F32, tag="rs_in")
                rs_out = dpool.tile([RPC, D], F32, tag="rs_out")
                nc.sync.dma_start(rs_in[:, :], pooled_full[:])
                nc.gpsimd.collective_compute(
                    "ReduceScatter", mybir.AluOpType.add,
                    replica_groups=[list(range(NCORES))],
                    ins=[rs_in[:, :]],
                    outs=[rs_out[:, :]],
                )
                pooled_sb = wpool.tile([RPC, D], F32, tag="pooled_sb")
                nc.sync.dma_start(pooled_sb[:], rs_out[:, :])

                # ---- MLP + softmax on own 8 rows ----
                with tc.tile_pool(name="ps_mlp", bufs=1,
                                  space="PSUM") as ps_mlp:
                    _mlp_tail(nc, tc, cpool, ps_mlp, pooled_sb, identity,
                              w1t_sb, b1_sb, w2t_sb, b2a_sb, b2b_sb,
                              w3a_sb, w3b_sb, b3_sb, out)

    nc.compile()
    return nc


def make_in_maps(x, emb, idf, W1, b1, W2, b2, W3, b3):
    xt = np.asarray(x, dtype=np.int32).T  # [B, S]
    # token layout per core: [p, r*16 + f] with s = p*16 + f
    xt4 = xt.reshape(B, P, STILES)  # [B, p, f]

    bf16 = mybir.dt.np(BF16)
    emb_pad = np.zeros((VPAD, D), dtype=np.float32)
    emb_pad[:V] = np.asarray(emb, dtype=np.float32)
    emb_bf16 = emb_pad.astype(bf16)

    idf_pad = np.zeros(VPAD, dtype=np.float32)
    idf_pad[:V] = np.asarray(idf, dtype=np.float32)
    idf_pad[0] = 0.0  # pad token contributes nothing
    idf_t = np.ascontiguousarray(idf_pad.reshape(NHI, NLO).T)  # [256, 200]
    idf_bf16 = idf_t.astype(bf16)

    w1t = np.ascontiguousarray(np.asarray(W1, dtype=np.float32).T)
    w2t = np.ascontiguousarray(np.asarray(W2, dtype=np.float32).T)
    w3t = np.ascontiguousarray(np.asarray(W3, dtype=np.float32).T)
    b1 = np.ascontiguousarray(np.asarray(b1, dtype=np.float32))
    b2 = np.ascontiguousarray(np.asarray(b2, dtype=np.float32))
    b3 = np.ascontiguousarray(np.asarray(b3, dtype=np.float32))

    in_maps = []
    for c in range(NCORES):
        rows = xt4[c * RPC : (c + 1) * RPC]  # [8, 128, 16]
        tok = np.ascontiguousarray(
            rows.transpose(1, 0, 2).reshape(P, NT)).astype(np.int32)
        m = {
            "xt": tok,
            "embs": np.ascontiguousarray(
                emb_bf16[c * VSH : (c + 1) * VSH]),
            "idf_t": idf_bf16,
            "w1t": w1t, "b1": b1, "w2t": w2t, "b2": b2,
            "w3t": w3t, "b3": b3,
        }
        in_maps.append(m)
    return in_maps



def _get_nc(reps=1):
    key = f"nc3_r{reps}"
    if key not in _CACHE:
        _CACHE[key] = _build_nc(reps)
    return _CACHE[key]


class _Runner:
    """Cached jitted shard_map over the NEFF custom call (mirrors
    bass2jax.run_bass_via_pjrt, but reusable with device-resident inputs)."""

    def __init__(self, nc):
        import jax
        from jax.experimental.shard_map import shard_map
        from jax.sharding import Mesh, NamedSharding, PartitionSpec

        from concourse import bass2jax

        bass2jax.install_neuronx_cc_hook()
        assert nc.dbg_addr is None
        partition_name = (
            nc.partition_id_tensor.name if nc.partition_id_tensor else None
        )
        self._nc = nc
        self._partition_name = partition_name

        self.jax = jax
        in_names, out_names, out_avals, zero_outs = [], [], [], []
        for alloc in nc.m.functions[0].allocations:
            if not isinstance(alloc, mybir.MemoryLocationSet):
                continue
            name = alloc.memorylocations[0].name
            if alloc.kind == "ExternalInput":
                if name == partition_name:
                    continue
                in_names.append(name)
            elif alloc.kind == "ExternalOutput":
                out_names.append(name)
                shape = tuple(alloc.tensor_shape)
                dtype = mybir.dt.np(alloc.dtype)
                out_avals.append(jax.core.ShapedArray(shape, dtype))
                zero_outs.append(np.zeros((NCORES * shape[0], *shape[1:]), dtype))
        self.in_names = list(in_names)
        self.out_names = out_names
        self.out_avals = out_avals
        self.zero_outs = zero_outs
        n_params = len(in_names)
        n_outs = len(out_names)
        bind_names = tuple(
            in_names + out_names + ([partition_name] if partition_name else [])
        )
        donate = tuple(range(n_params, n_params + n_outs))

        def _body(*args):
            operands = list(args)
            if partition_name is not None:
                operands.append(bass2jax.partition_id_tensor())
            outs = bass2jax._bass_exec_p.bind(
                *operands,
                out_avals=tuple(out_avals),
                in_names=bind_names,
                out_names=tuple(out_names),
                lowering_input_output_aliases=(),
                sim_require_finite=True,
                sim_require_nnan=True,
                nc=nc,
            )
            return tuple(outs)

        devices = jax.devices()[:NCORES]
        self.mesh = Mesh(np.asarray(devices), ("core",))
        self.sharding = NamedSharding(self.mesh, PartitionSpec("core"))
        in_specs = (PartitionSpec("core"),) * (n_params + n_outs)
        out_specs = (PartitionSpec("core"),) * n_outs
        self.fn = jax.jit(
            shard_map(
                _body,
                mesh=self.mesh,
                in_specs=in_specs,
                out_specs=out_specs,
                check_rep=False,
            ),
            donate_argnums=donate,
            keep_unused=True,
        )

    def put_inputs(self, in_maps):
        concat = [
            np.concatenate([np.asarray(m[name]) for m in in_maps], axis=0)
            for name in self.in_names
        ]
        return [self.jax.device_put(a, self.sharding) for a in concat]

    def run(self, dev_in):
        zo = [self.jax.device_put(z, self.sharding) for z in self.zero_outs]
        outs = self.fn(*dev_in, *zo)
        self.jax.block_until_ready(outs)
        return outs

    def run_np(self, dev_in):
        outs = self.run(dev_in)
        return {
            name: np.asarray(outs[i]).reshape(NCORES, *self.out_avals[i].shape)
            for i, name in enumerate(self.out_names)
        }




def _get_runner(reps=1):
    key = f"runner3_r{reps}"
    if key not in _CACHE:
        _CACHE[key] = _Runner(_get_nc(reps))
    return _CACHE[key]


def kernel(x, emb, idf, W1, b1, W2, b2, W3, b3):
    in_maps = make_in_maps(x, emb, idf, W1, b1, W2, b2, W3, b3)
    runner = _get_runner(1)
    dev_in = runner.put_inputs(in_maps)
    outs = runner.run_np(dev_in)
    outp = np.concatenate([outs["out"][c] for c in range(NCORES)], axis=0)
    return outp.astype(np.float32)



# revision 3
# speedup vs baseline: 5.3097x; 5.3097x over previous
"""Trainium2 Bass kernel for tf-idf embedding pooling + MLP (v2).

Math identity: pooled[b] = sum_v c_{b,v}^2 * idf_v * emb_v where c = per-row
token counts. v = hi*128 + lo (radix 128 x 392). Per-core histogram of its 8
batch rows via one-hot matmuls H[lo, hi] += onehot_lo^T @ onehot_hi, then
a = H^2*idf, AllToAll redistributes a by vocab shard (hi ranges), each core
contracts its 6272-row bf16 emb shard for all 64 rows, ReduceScatter returns
final pooled rows, tiny MLP + softmax per core.

v2 vs v1: the one-hots are precomputed on host in fp8 and DMA'd in (the HW
profile showed DVE/Pool is_equal one-hot generation at 1.5-3.9us per tile,
~250us/rep — 85% of the runtime). Radix 128x392 halves the matmul count
(one LDW+MM per token tile instead of two). Everything else (collectives,
pooled contraction, MLP tail) is unchanged in structure.
"""

import sys

import numpy as np

sys.path.insert(0, "/opt/trn_rl_repo")

import concourse.bass as bass  # noqa: E402,F401
import concourse.mybir as mybir  # noqa: E402
import concourse.tile as tile  # noqa: E402
from concourse import bacc  # noqa: E402
from concourse.masks import make_identity  # noqa: E402

P = 128
S = 2048
B = 64
D = 256
V = 50000
NCORES = 8
RPC = B // NCORES  # 8 rows per core
NLO = 128
NHI = 392  # 391 real hi values + 1 pad (tokens never land there)
NHL = NHI // NCORES  # 49 hi rows per vocab shard
VSH = NHL * NLO  # 6272 vocab rows per shard
VPAD = NHI * NLO  # 50176
STILES = S // P  # 16
NT = RPC * STILES  # 128 token tiles of 128

F32 = mybir.dt.float32
BF16 = mybir.dt.bfloat16
F8 = mybir.dt.float8e4

_CACHE = {}


def _mlp_tail(nc, tc, cpool, ps_mlp, pooled_sb, identity,
              w1t_sb, b1_sb, w2t_sb, b2a_sb, b2b_sb, w3a_sb, w3b_sb, b3_sb, out):
    """pooled_sb [RPC, 256] f32 -> softmax out DMA."""
    pooledT = cpool.tile([P, 2, RPC], F32, tag="pooledT", bufs=2)
    for kc in range(2):
        ptp = ps_mlp.tile([P, RPC], F32, tag="ptp")
        nc.tensor.transpose(
            ptp[:, :], pooled_sb[:, kc * P : (kc + 1) * P], identity[:RPC, :RPC]
        )
        nc.vector.tensor_copy(pooledT[:, kc, :], ptp[:, :])

    h1_ps = ps_mlp.tile([100, RPC], F32, tag="h1")
    for kc in range(2):
        nc.tensor.matmul(
            h1_ps[:, :], lhsT=w1t_sb[:, kc, :], rhs=pooledT[:, kc, :],
            start=(kc == 0), stop=(kc == 1),
        )
    h1_sb = cpool.tile([100, RPC], F32, tag="h1_sb", bufs=2)
    nc.scalar.activation(
        h1_sb[:], h1_ps[:, :], mybir.ActivationFunctionType.Relu,
        bias=b1_sb[:, 0:1], scale=1.0,
    )

    h2a_ps = ps_mlp.tile([P, RPC], F32, tag="h2a")
    nc.tensor.matmul(h2a_ps[:, :], lhsT=w2t_sb[:, 0:128], rhs=h1_sb[:, :],
                     start=True, stop=True)
    h2b_ps = ps_mlp.tile([22, RPC], F32, tag="h2b")
    nc.tensor.matmul(h2b_ps[:, :], lhsT=w2t_sb[:, 128:150], rhs=h1_sb[:, :],
                     start=True, stop=True)
    h2a_sb = cpool.tile([P, RPC], F32, tag="h2a_sb", bufs=2)
    h2b_sb = cpool.tile([22, RPC], F32, tag="h2b_sb", bufs=2)
    nc.scalar.activation(h2a_sb[:], h2a_ps[:, :],
                         mybir.ActivationFunctionType.Relu,
                         bias=b2a_sb[:, 0:1], scale=1.0)
    nc.scalar.activation(h2b_sb[:], h2b_ps[:, :],
                         mybir.ActivationFunctionType.Relu,
                         bias=b2b_sb[:, 0:1], scale=1.0)

    lg_ps = ps_mlp.tile([2, RPC], F32, tag="lg")
    nc.tensor.matmul(lg_ps[:, :], lhsT=w3a_sb[:, :], rhs=h2a_sb[:, :],
                     start=True, stop=False)
    nc.tensor.matmul(lg_ps[:, :], lhsT=w3b_sb[:, :], rhs=h2b_sb[:, :],
                     start=False, stop=True)
    lg_sb = cpool.tile([2, RPC], F32, tag="lg_sb", bufs=2)
    nc.scalar.add(lg_sb[:], lg_ps[:, :], b3_sb[:, 0:1])

    lt_ps = ps_mlp.tile([RPC, 2], F32, tag="lt")
    nc.tensor.transpose(lt_ps[:, :], lg_sb[:, :], identity[:2, :2])
    e_sb = cpool.tile([RPC, 2], F32, tag="e_sb", bufs=2)
    nc.scalar.activation(e_sb[:], lt_ps[:, :], mybir.ActivationFunctionType.Exp)
    ssum = cpool.tile([RPC, 1], F32, tag="ssum", bufs=2)
    nc.vector.tensor_reduce(ssum[:], e_sb[:], axis=mybir.AxisListType.X,
                            op=mybir.AluOpType.add)
    rinv = cpool.tile([RPC, 1], F32, tag="rinv", bufs=2)
    nc.vector.reciprocal(rinv[:], ssum[:])
    res_sb = cpool.tile([RPC, 2], F32, tag="res_sb", bufs=2)
    nc.vector.tensor_scalar(out=res_sb[:], in0=e_sb[:], scalar1=rinv[:, 0:1],
                            scalar2=None, op0=mybir.AluOpType.mult)
    nc.sync.dma_start(out[:, :], res_sb[:])


def _build_nc(reps=1):
    nc = bacc.Bacc(None, target_bir_lowering=False, debug=False)

    ohlo = nc.dram_tensor("ohlo", [P, RPC, STILES, NLO], F8, kind="ExternalInput")
    ohhi = nc.dram_tensor("ohhi", [P, RPC, STILES, NHI], F8, kind="ExternalInput")
    embs = nc.dram_tensor("embs", [VSH, D], BF16, kind="ExternalInput")
    idf_t = nc.dram_tensor("idf_t", [NLO, NHI], BF16, kind="ExternalInput")
    w1t = nc.dram_tensor("w1t", [256, 100], F32, kind="ExternalInput")
    b1 = nc.dram_tensor("b1", [100], F32, kind="ExternalInput")
    w2t = nc.dram_tensor("w2t", [100, 150], F32, kind="ExternalInput")
    b2 = nc.dram_tensor("b2", [150], F32, kind="ExternalInput")
    w3t = nc.dram_tensor("w3t", [150, 2], F32, kind="ExternalInput")
    b3 = nc.dram_tensor("b3", [2], F32, kind="ExternalInput")
    out = nc.dram_tensor("out", [RPC, 2], F32, kind="ExternalOutput")

    with tile.TileContext(nc) as tc:
        with (
            tc.tile_pool(name="const", bufs=1) as cpool,
            tc.tile_pool(name="oh_lo", bufs=2) as lopool,
            tc.tile_pool(name="oh_hi", bufs=2) as hipool,
            tc.tile_pool(name="emb", bufs=1) as epool,
            tc.tile_pool(name="work", bufs=2) as wpool,
            tc.tile_pool(name="sq", bufs=4) as sqpool,
            tc.tile_pool(name="dram", bufs=2, space="DRAM") as dpool,
            tc.tile_pool(name="ps_acc", bufs=2, space="PSUM") as ps_acc,
        ):
            # ---------- constants (amortized across reps, same as v1) ----------
            identity = cpool.tile([P, P], F32)
            make_identity(nc, identity[:])

            idf_sb = cpool.tile([P, NHI], BF16)
            nc.sync.dma_start(idf_sb[:, :], idf_t[:, :])

            w1t_sb = cpool.tile([P, 2, 100], F32)
            nc.sync.dma_start(w1t_sb[:, :, :],
                              w1t[:, :].rearrange("(c p) m -> p c m", p=P))
            b1_sb = cpool.tile([100, 1], F32)
            nc.sync.dma_start(b1_sb[:, :], b1[:, None])
            w2t_sb = cpool.tile([100, 150], F32)
            nc.sync.dma_start(w2t_sb[:, :], w2t[:, :])
            b2a_sb = cpool.tile([128, 1], F32)
            b2b_sb = cpool.tile([22, 1], F32)
            nc.sync.dma_start(b2a_sb[:, :], b2[:128, None])
            nc.sync.dma_start(b2b_sb[:, :], b2[128:150, None])
            w3a_sb = cpool.tile([128, 2], F32)
            w3b_sb = cpool.tile([22, 2], F32)
            nc.sync.dma_start(w3a_sb[:, :], w3t[0:128, :])
            nc.sync.dma_start(w3b_sb[:, :], w3t[128:150, :])
            b3_sb = cpool.tile([2, 1], F32)
            nc.sync.dma_start(b3_sb[:, :], b3[:, None])

            for _rep in range(reps):
                # ---- per-row one-hot DMAs (row r's matmuls depend only on
                # its own tiles, so histogram overlaps the later rows' DMA) ----
                lo_sb = []
                hi_sb = []
                for r in range(RPC):
                    lt = lopool.tile([P, STILES, NLO], F8, tag=f"lo{r}")
                    ht = hipool.tile([P, STILES, NHI], F8, tag=f"hi{r}")
                    nc.sync.dma_start(lt[:, :, :], ohlo[:, r, :, :])
                    nc.sync.dma_start(ht[:, :, :], ohhi[:, r, :, :])
                    lo_sb.append(lt)
                    hi_sb.append(ht)

                # ---- emb shard preload (needed only at the pooled matmul) ----
                emb_sb = epool.tile([P, VSH // P, D], BF16, tag="emb_sb")
                nc.sync.dma_start(
                    emb_sb[:, :, :],
                    embs[:, :].rearrange("(c p) d -> p c d", p=P),
                )

                # ---- per-row histograms -> a = H^2 * idf (bf16) ----
                # a_sb layout [p, hi, r]: per (dst_shard) the (hl, r) block is
                # 49*8=392 contiguous elements (784B DMA runs for the a2a).
                a_sb = wpool.tile([P, NHI, RPC], BF16, tag="a_sb")

                with tc.tile_pool(name="ps_ht", bufs=2, space="PSUM") as ps_ht:
                    for r in range(RPC):
                        ht_ps = ps_ht.tile([P, NHI], F32, tag="ht")
                        for f in range(STILES):
                            nc.tensor.matmul(
                                ht_ps[:, :],
                                lhsT=lo_sb[r][:, f, :],
                                rhs=hi_sb[r][:, f, :],
                                start=(f == 0), stop=(f == STILES - 1))
                        sq = sqpool.tile([P, NHI], BF16, tag="sq")
                        nc.scalar.square(sq[:], ht_ps[:, :])
                        tt_eng = nc.vector if r % 2 else nc.gpsimd
                        tt_eng.tensor_tensor(
                            out=a_sb[:, :, r], in0=sq[:], in1=idf_sb[:, :],
                            op=mybir.AluOpType.mult)

                # ---- AllToAll: a2a[dst][p][hl][r] bf16 ----
                a2a_in = dpool.tile([NCORES, P, NHL, RPC], BF16, tag="a2a_in")
                a2a_out = dpool.tile([NCORES, P, NHL, RPC], BF16, tag="a2a_out")
                nc.sync.dma_start(
                    a2a_in[:, :, :, :].rearrange("dst p hl r -> p dst (hl r)"),
                    a_sb[:, :, :].rearrange("p (dst hl) r -> p dst (hl r)",
                                            dst=NCORES),
                )
                nc.gpsimd.collective_compute(
                    "AllToAll", mybir.AluOpType.bypass,
                    replica_groups=[list(range(NCORES))],
                    ins=[a2a_in[:, :, :, :]],
                    outs=[a2a_out[:, :, :, :]],
                )
                # receive into [p][src][hl][r] (784B contiguous runs)
                recv_sb = wpool.tile([P, NCORES, NHL, RPC], BF16, tag="recv_sb")
                nc.sync.dma_start(
                    recv_sb[:, :, :, :].rearrange("p src hl r -> p src (hl r)"),
                    a2a_out[:, :, :, :].rearrange("src p hl r -> p src (hl r)"),
                )
                # matmul weights need ONE free dim: reshuffle to
                # [p][hl][(src r)] split across the mostly-idle ACT + engines
                recv_mm = wpool.tile([P, NHL, NCORES, RPC], BF16, tag="recv_mm")
                HLH = NHL // 2 + 1  # 25 | 24 split
                nc.scalar.activation(
                    recv_mm[:, :HLH, :, :],
                    recv_sb[:, :, :HLH, :].rearrange("p src hl r -> p hl src r"),
                    mybir.ActivationFunctionType.Copy, scale=1.0)
                nc.vector.tensor_copy(
                    recv_mm[:, HLH:, :, :],
                    recv_sb[:, :, HLH:, :].rearrange("p src hl r -> p hl src r"))

                # ---- pooled[64, 256] partial = sum over my vocab shard ----
                pooled_ps = ps_acc.tile([B, D], F32, tag="pooled")
                for c in range(VSH // P):  # 49 chunks of 128 vocab rows
                    nc.tensor.matmul(
                        pooled_ps[:, :],
                        lhsT=recv_mm[:, c, :, :].rearrange("p src r -> p (src r)"),
                        rhs=emb_sb[:, c, :],
                        start=(c == 0), stop=(c == VSH // P - 1))
                pooled_full = wpool.tile([B, D], F32, tag="pooled_full")
                nc.vector.tensor_copy(pooled_full[:], pooled_ps[:, :])

                # ---- ReduceScatter -> my 8 rows ----
                rs_in = dpool.tile([B, D], F32, tag="rs_in")
                rs_out = dpool.tile([RPC, D], F32, tag="rs_out")
                nc.sync.dma_start(rs_in[:, :], pooled_full[:])
                nc.gpsimd.collective_compute(
                    "ReduceScatter", mybir.AluOpType.add,
                    replica_groups=[list(range(NCORES))],
                    ins=[rs_in[:, :]],
                    outs=[rs_out[:, :]],
                )
                pooled_sb = wpool.tile([RPC, D], F32, tag="pooled_sb")
                nc.sync.dma_start(pooled_sb[:], rs_out[:, :])

                # ---- MLP + softmax on own 8 rows ----
                with tc.tile_pool(name="ps_mlp", bufs=1,
                                  space="PSUM") as ps_mlp:
                    _mlp_tail(nc, tc, cpool, ps_mlp, pooled_sb, identity,
                              w1t_sb, b1_sb, w2t_sb, b2a_sb, b2b_sb,
                              w3a_sb, w3b_sb, b3_sb, out)

    nc.compile()
    return nc


def make_in_maps(x, emb, idf, W1, b1, W2, b2, W3, b3):
    bf16 = mybir.dt.np(BF16)
    f8 = mybir.dt.np(F8)

    xt = np.asarray(x, dtype=np.int64).T  # [B, S]
    # token layout per core: tile t = r*STILES + f holds tokens s = p*16 + f
    xt4 = xt.reshape(B, P, STILES)  # [b, p, f]

    emb_pad = np.zeros((VPAD, D), dtype=np.float32)
    emb_pad[:V] = np.asarray(emb, dtype=np.float32)
    emb_bf16 = emb_pad.astype(bf16)

    idf_pad = np.zeros(VPAD, dtype=np.float32)
    idf_pad[:V] = np.asarray(idf, dtype=np.float32)
    idf_pad[0] = 0.0  # pad token contributes nothing
    idf_t = np.ascontiguousarray(idf_pad.reshape(NHI, NLO).T)  # [128, 392]
    idf_bf16 = idf_t.astype(bf16)

    w1t = np.ascontiguousarray(np.asarray(W1, dtype=np.float32).T)
    w2t = np.ascontiguousarray(np.asarray(W2, dtype=np.float32).T)
    w3t = np.ascontiguousarray(np.asarray(W3, dtype=np.float32).T)
    b1 = np.ascontiguousarray(np.asarray(b1, dtype=np.float32))
    b2 = np.ascontiguousarray(np.asarray(b2, dtype=np.float32))
    b3 = np.ascontiguousarray(np.asarray(b3, dtype=np.float32))

    pp = np.arange(P)[:, None, None]  # [P, 1, 1]
    rr = np.arange(RPC)[None, :, None]
    ff = np.arange(STILES)[None, None, :]

    in_maps = []
    for c in range(NCORES):
        rows = xt4[c * RPC : (c + 1) * RPC]  # [r=8, p=128, f=16]
        tok = rows.transpose(1, 0, 2)  # [p, r, f]
        lo = (tok & (NLO - 1)).astype(np.int64)
        hi = (tok >> 7).astype(np.int64)
        ohlo = np.zeros((P, RPC, STILES, NLO), dtype=f8)
        ohhi = np.zeros((P, RPC, STILES, NHI), dtype=f8)
        one = f8(1.0)
        ohlo[pp, rr, ff, lo] = one
        ohhi[pp, rr, ff, hi] = one
        m = {
            "ohlo": ohlo,
            "ohhi": ohhi,
            "embs": np.ascontiguousarray(emb_bf16[c * VSH : (c + 1) * VSH]),
            "idf_t": idf_bf16,
            "w1t": w1t, "b1": b1, "w2t": w2t, "b2": b2,
            "w3t": w3t, "b3": b3,
        }
        in_maps.append(m)
    return in_maps


def _get_nc(reps=1):
    key = f"nc4_r{reps}"
    if key not in _CACHE:
        _CACHE[key] = _build_nc(reps)
    return _CACHE[key]


class _Runner:
    """Cached jitted shard_map over the NEFF custom call (mirrors
    bass2jax.run_bass_via_pjrt, but reusable with device-resident inputs)."""

    def __init__(self, nc):
        import jax
        from jax.experimental.shard_map import shard_map
        from jax.sharding import Mesh, NamedSharding, PartitionSpec

        from concourse import bass2jax

        bass2jax.install_neuronx_cc_hook()
        assert nc.dbg_addr is None
        partition_name = (
            nc.partition_id_tensor.name if nc.partition_id_tensor else None
        )
        self._nc = nc
        self._partition_name = partition_name

        self.jax = jax
        in_names, out_names, out_avals, zero_outs = [], [], [], []
        for alloc in nc.m.functions[0].allocations:
            if not isinstance(alloc, mybir.MemoryLocationSet):
                continue
            name = alloc.memorylocations[0].name
            if alloc.kind == "ExternalInput":
                if name == partition_name:
                    continue
                in_names.append(name)
            elif alloc.kind == "ExternalOutput":
                out_names.append(name)
                shape = tuple(alloc.tensor_shape)
                dtype = mybir.dt.np(alloc.dtype)
                out_avals.append(jax.core.ShapedArray(shape, dtype))
                zero_outs.append(np.zeros((NCORES * shape[0], *shape[1:]), dtype))
        self.in_names = list(in_names)
        self.out_names = out_names
        self.out_avals = out_avals
        self.zero_outs = zero_outs
        n_params = len(in_names)
        n_outs = len(out_names)
        bind_names = tuple(
            in_names + out_names + ([partition_name] if partition_name else [])
        )
        donate = tuple(range(n_params, n_params + n_outs))

        def _body(*args):
            operands = list(args)
            if partition_name is not None:
                operands.append(bass2jax.partition_id_tensor())
            outs = bass2jax._bass_exec_p.bind(
                *operands,
                out_avals=tuple(out_avals),
                in_names=bind_names,
                out_names=tuple(out_names),
                lowering_input_output_aliases=(),
                sim_require_finite=True,
                sim_require_nnan=True,
                nc=nc,
            )
            return tuple(outs)

        devices = jax.devices()[:NCORES]
        self.mesh = Mesh(np.asarray(devices), ("core",))
        self.sharding = NamedSharding(self.mesh, PartitionSpec("core"))
        in_specs = (PartitionSpec("core"),) * (n_params + n_outs)
        out_specs = (PartitionSpec("core"),) * n_outs
        self.fn = jax.jit(
            shard_map(
                _body,
                mesh=self.mesh,
                in_specs=in_specs,
                out_specs=out_specs,
                check_rep=False,
            ),
            donate_argnums=donate,
            keep_unused=True,
        )

    def put_inputs(self, in_maps):
        concat = [
            np.concatenate([np.asarray(m[name]) for m in in_maps], axis=0)
            for name in self.in_names
        ]
        return [self.jax.device_put(a, self.sharding) for a in concat]

    def run(self, dev_in):
        zo = [self.jax.device_put(z, self.sharding) for z in self.zero_outs]
        outs = self.fn(*dev_in, *zo)
        self.jax.block_until_ready(outs)
        return outs

    def run_np(self, dev_in):
        outs = self.run(dev_in)
        return {
            name: np.asarray(outs[i]).reshape(NCORES, *self.out_avals[i].shape)
            for i, name in enumerate(self.out_names)
        }


def _get_runner(reps=1):
    key = f"runner4_r{reps}"
    if key not in _CACHE:
        _CACHE[key] = _Runner(_get_nc(reps))
    return _CACHE[key]


def kernel(x, emb, idf, W1, b1, W2, b2, W3, b3):
    in_maps = make_in_maps(x, emb, idf, W1, b1, W2, b2, W3, b3)
    runner = _get_runner(1)
    dev_in = runner.put_inputs(in_maps)
    outs = runner.run_np(dev_in)
    outp = np.concatenate([outs["out"][c] for c in range(NCORES)], axis=0)
    return outp.astype(np.float32)


# revision 5
# speedup vs baseline: 72.3504x; 13.6260x over previous
"""Trainium2 Bass kernel for tf-idf embedding pooling + MLP (v3).

Math identity: pooled[b] = sum_v c_{b,v}^2 * idf_v * emb_v where c = per-row
token counts. v = hi*128 + lo (radix 128 x 392). Per-core histogram of its 8
batch rows via one-hot matmuls H[lo, hi] += onehot_lo^T @ onehot_hi (one-hots
precomputed on host in fp8 — on-device is_equal generation measured 10-30x
slower than nominal DVE rate). AllToAll redistributes raw counts H by vocab
shard (hi ranges) in fp8 (counts <= 8 are exact in e4m3), receiving core
squares them fused with the reshuffle, then contracts its 6272-row bf16
idf-prescaled emb shard (emb*idf folded on host) for all 64 rows;
ReduceScatter returns final pooled rows; tiny MLP + softmax per core.

The rep loop is software-pipelined 3 deep (front: DMAs+hist+a2a | mid:
square+pooled+RS | tail: MLP+out) so rep k+1's histogram matmuls overlap rep
k's collectives — this also keeps the PE HAM-warm (the v2 profile showed the
whole histogram running at the cold 1.2 GHz clock because the ~30us
collective gap re-throttled the PE every rep).
"""

import sys

import numpy as np

sys.path.insert(0, "/opt/trn_rl_repo")

import concourse.bass as bass  # noqa: E402,F401
import concourse.mybir as mybir  # noqa: E402
import concourse.tile as tile  # noqa: E402
from concourse import bacc  # noqa: E402
from concourse.masks import make_identity  # noqa: E402

P = 128
S = 2048
B = 64
D = 256
V = 50000
NCORES = 8
RPC = B // NCORES  # 8 rows per core
NLO = 128
NHI = 392  # 391 real hi values + 1 pad (tokens never land there)
NHL = NHI // NCORES  # 49 hi rows per vocab shard
VSH = NHL * NLO  # 6272 vocab rows per shard
VPAD = NHI * NLO  # 50176
STILES = S // P  # 16
NT = RPC * STILES  # 128 token tiles of 128

F32 = mybir.dt.float32
BF16 = mybir.dt.bfloat16
F8 = mybir.dt.float8e4

_CACHE = {}


def _mlp_tail(nc, tc, cpool, ps_mlp, pooled_sb, identity,
              w1t_sb, b1_sb, w2t_sb, b2a_sb, b2b_sb, w3a_sb, w3b_sb, b3_sb, out):
    """pooled_sb [RPC, 256] f32 -> softmax out DMA.

    All PSUM intermediates are disjoint column slices of one 512-col bank
    (PSUM pool allocation is bank-granular; 6 separate tags would need 6
    banks)."""
    mlp_ps = ps_mlp.tile([P, 64], F32, tag="mlp")
    pooledT = cpool.tile([P, 2, RPC], F32, tag="pooledT", bufs=2)
    for kc in range(2):
        ptp = mlp_ps[:, kc * RPC : (kc + 1) * RPC]
        nc.tensor.transpose(
            ptp, pooled_sb[:, kc * P : (kc + 1) * P], identity[:RPC, :RPC]
        )
        nc.vector.tensor_copy(pooledT[:, kc, :], ptp)

    h1_ps = mlp_ps[:100, 16:24]
    for kc in range(2):
        nc.tensor.matmul(
            h1_ps, lhsT=w1t_sb[:, kc, :], rhs=pooledT[:, kc, :],
            start=(kc == 0), stop=(kc == 1),
        )
    h1_sb = cpool.tile([100, RPC], F32, tag="h1_sb", bufs=2)
    nc.scalar.activation(
        h1_sb[:], h1_ps, mybir.ActivationFunctionType.Relu,
        bias=b1_sb[:, 0:1], scale=1.0,
    )

    h2a_ps = mlp_ps[:, 24:32]
    nc.tensor.matmul(h2a_ps, lhsT=w2t_sb[:, 0:128], rhs=h1_sb[:, :],
                     start=True, stop=True)
    h2b_ps = mlp_ps[:22, 32:40]
    nc.tensor.matmul(h2b_ps, lhsT=w2t_sb[:, 128:150], rhs=h1_sb[:, :],
                     start=True, stop=True)
    h2a_sb = cpool.tile([P, RPC], F32, tag="h2a_sb", bufs=2)
    h2b_sb = cpool.tile([22, RPC], F32, tag="h2b_sb", bufs=2)
    nc.scalar.activation(h2a_sb[:], h2a_ps,
                         mybir.ActivationFunctionType.Relu,
                         bias=b2a_sb[:, 0:1], scale=1.0)
    nc.scalar.activation(h2b_sb[:], h2b_ps,
                         mybir.ActivationFunctionType.Relu,
                         bias=b2b_sb[:, 0:1], scale=1.0)

    lg_ps = mlp_ps[:2, 40:48]
    nc.tensor.matmul(lg_ps, lhsT=w3a_sb[:, :], rhs=h2a_sb[:, :],
                     start=True, stop=False)
    nc.tensor.matmul(lg_ps, lhsT=w3b_sb[:, :], rhs=h2b_sb[:, :],
                     start=False, stop=True)
    lg_sb = cpool.tile([2, RPC], F32, tag="lg_sb", bufs=2)
    nc.scalar.add(lg_sb[:], lg_ps, b3_sb[:, 0:1])

    lt_ps = mlp_ps[:RPC, 48:50]
    nc.tensor.transpose(lt_ps, lg_sb[:, :], identity[:2, :2])
    e_sb = cpool.tile([RPC, 2], F32, tag="e_sb", bufs=2)
    nc.scalar.activation(e_sb[:], lt_ps[:, :], mybir.ActivationFunctionType.Exp)
    ssum = cpool.tile([RPC, 1], F32, tag="ssum", bufs=2)
    nc.vector.tensor_reduce(ssum[:], e_sb[:], axis=mybir.AxisListType.X,
                            op=mybir.AluOpType.add)
    rinv = cpool.tile([RPC, 1], F32, tag="rinv", bufs=2)
    nc.vector.reciprocal(rinv[:], ssum[:])
    res_sb = cpool.tile([RPC, 2], F32, tag="res_sb", bufs=2)
    nc.vector.tensor_scalar(out=res_sb[:], in0=e_sb[:], scalar1=rinv[:, 0:1],
                            scalar2=None, op0=mybir.AluOpType.mult)
    nc.sync.dma_start(out[:, :], res_sb[:])


def _build_nc(reps=1):
    nc = bacc.Bacc(None, target_bir_lowering=False, debug=False)

    ohlo = nc.dram_tensor("ohlo", [P, RPC, STILES, NLO], F8, kind="ExternalInput")
    ohhi = nc.dram_tensor("ohhi", [P, RPC, STILES, NHI], F8, kind="ExternalInput")
    embs = nc.dram_tensor("embs", [VSH, D], BF16, kind="ExternalInput")
    w1t = nc.dram_tensor("w1t", [256, 100], F32, kind="ExternalInput")
    b1 = nc.dram_tensor("b1", [100], F32, kind="ExternalInput")
    w2t = nc.dram_tensor("w2t", [100, 150], F32, kind="ExternalInput")
    b2 = nc.dram_tensor("b2", [150], F32, kind="ExternalInput")
    w3t = nc.dram_tensor("w3t", [150, 2], F32, kind="ExternalInput")
    b3 = nc.dram_tensor("b3", [2], F32, kind="ExternalInput")
    out = nc.dram_tensor("out", [RPC, 2], F32, kind="ExternalOutput")

    with tile.TileContext(nc) as tc:
        with (
            tc.tile_pool(name="const", bufs=1) as cpool,
            tc.tile_pool(name="oh_lo", bufs=2) as lopool,
            tc.tile_pool(name="oh_hi", bufs=2) as hipool,
            tc.tile_pool(name="emb", bufs=1) as epool,
            tc.tile_pool(name="work", bufs=2) as wpool,
            tc.tile_pool(name="sq", bufs=4) as sqpool,
            tc.tile_pool(name="dram", bufs=2, space="DRAM") as dpool,
            tc.tile_pool(name="ps_ht", bufs=2, space="PSUM") as ps_ht,
            tc.tile_pool(name="ps_acc", bufs=2, space="PSUM") as ps_acc,
            tc.tile_pool(name="ps_mlp", bufs=1, space="PSUM") as ps_mlp,
        ):
            # ---------- constants (amortized across reps, same as v1) ----------
            identity = cpool.tile([P, P], F32)
            make_identity(nc, identity[:])

            w1t_sb = cpool.tile([P, 2, 100], F32)
            nc.sync.dma_start(w1t_sb[:, :, :],
                              w1t[:, :].rearrange("(c p) m -> p c m", p=P))
            b1_sb = cpool.tile([100, 1], F32)
            nc.sync.dma_start(b1_sb[:, :], b1[:, None])
            w2t_sb = cpool.tile([100, 150], F32)
            nc.sync.dma_start(w2t_sb[:, :], w2t[:, :])
            b2a_sb = cpool.tile([128, 1], F32)
            b2b_sb = cpool.tile([22, 1], F32)
            nc.sync.dma_start(b2a_sb[:, :], b2[:128, None])
            nc.sync.dma_start(b2b_sb[:, :], b2[128:150, None])
            w3a_sb = cpool.tile([128, 2], F32)
            w3b_sb = cpool.tile([22, 2], F32)
            nc.sync.dma_start(w3a_sb[:, :], w3t[0:128, :])
            nc.sync.dma_start(w3b_sb[:, :], w3t[128:150, :])
            b3_sb = cpool.tile([2, 1], F32)
            nc.sync.dma_start(b3_sb[:, :], b3[:, None])

            def emit_front(rep):
                """DMAs + per-row histograms + fp8 count AllToAll."""
                lo_sb, hi_sb = [], []
                for r in range(RPC):
                    lt = lopool.tile([P, STILES, NLO], F8, tag=f"lo{r}")
                    ht = hipool.tile([P, STILES, NHI], F8, tag=f"hi{r}")
                    nc.sync.dma_start(lt[:, :, :], ohlo[:, r, :, :])
                    nc.sync.dma_start(ht[:, :, :], ohhi[:, r, :, :])
                    lo_sb.append(lt)
                    hi_sb.append(ht)

                emb_sb = epool.tile([P, VSH // P, D], BF16, tag="emb_sb")
                nc.sync.dma_start(
                    emb_sb[:, :, :],
                    embs[:, :].rearrange("(c p) d -> p c d", p=P),
                )

                # a_sb layout [p, hi, r] fp8 raw counts: per dst_shard the
                # (hl, r) block is 49*8=392 contiguous bytes for the a2a.
                a_sb = wpool.tile([P, NHI, RPC], F8, tag="a_sb")
                for r in range(RPC):
                    ht_ps = ps_ht.tile([P, NHI], F32, tag="ht")
                    for f in range(STILES):
                        nc.tensor.matmul(
                            ht_ps[:, :],
                            lhsT=lo_sb[r][:, f, :],
                            rhs=hi_sb[r][:, f, :],
                            start=(f == 0), stop=(f == STILES - 1))
                    cp_eng = nc.vector if r % 2 else nc.scalar
                    if cp_eng is nc.scalar:
                        nc.scalar.copy(a_sb[:, :, r], ht_ps[:, :])
                    else:
                        nc.vector.tensor_copy(a_sb[:, :, r], ht_ps[:, :])

                a2a_in = dpool.tile([NCORES, P, NHL, RPC], F8, tag="a2a_in")
                a2a_out = dpool.tile([NCORES, P, NHL, RPC], F8, tag="a2a_out")
                nc.sync.dma_start(
                    a2a_in[:, :, :, :].rearrange("dst p hl r -> p dst (hl r)"),
                    a_sb[:, :, :].rearrange("p (dst hl) r -> p dst (hl r)",
                                            dst=NCORES),
                )
                nc.gpsimd.collective_compute(
                    "AllToAll", mybir.AluOpType.bypass,
                    replica_groups=[list(range(NCORES))],
                    ins=[a2a_in[:, :, :, :]],
                    outs=[a2a_out[:, :, :, :]],
                )
                return {"a2a_out": a2a_out, "emb_sb": emb_sb}

            def emit_mid(st):
                """recv + fused square/reshuffle + pooled matmul + RS."""
                recv_sb = wpool.tile([P, NCORES, NHL, RPC], F8, tag="recv_sb")
                nc.sync.dma_start(
                    recv_sb[:, :, :, :].rearrange("p src hl r -> p src (hl r)"),
                    st["a2a_out"][:, :, :, :].rearrange(
                        "src p hl r -> p src (hl r)"),
                )
                # square the counts fused with the [p][hl][(src r)] reshuffle
                recv_mm = wpool.tile([P, NHL, NCORES, RPC], BF16, tag="recv_mm")
                HLH = NHL // 2 + 1  # 25 | 24 split across ACT / DVE
                nc.scalar.activation(
                    recv_mm[:, :HLH, :, :],
                    recv_sb[:, :, :HLH, :].rearrange("p src hl r -> p hl src r"),
                    mybir.ActivationFunctionType.Square, scale=1.0)
                dve_in = recv_sb[:, :, HLH:, :].rearrange(
                    "p src hl r -> p hl src r")
                nc.vector.tensor_tensor(
                    out=recv_mm[:, HLH:, :, :], in0=dve_in, in1=dve_in,
                    op=mybir.AluOpType.mult)

                pooled_ps = ps_acc.tile([B, D], F32, tag="pooled")
                emb_sb = st["emb_sb"]
                for c in range(VSH // P):  # 49 chunks of 128 vocab rows
                    nc.tensor.matmul(
                        pooled_ps[:, :],
                        lhsT=recv_mm[:, c, :, :].rearrange(
                            "p src r -> p (src r)"),
                        rhs=emb_sb[:, c, :],
                        start=(c == 0), stop=(c == VSH // P - 1))
                pooled_full = wpool.tile([B, D], F32, tag="pooled_full")
                nc.vector.tensor_copy(pooled_full[:], pooled_ps[:, :])

                rs_in = dpool.tile([B, D], F32, tag="rs_in")
                rs_out = dpool.tile([RPC, D], F32, tag="rs_out")
                nc.sync.dma_start(rs_in[:, :], pooled_full[:])
                nc.gpsimd.collective_compute(
                    "ReduceScatter", mybir.AluOpType.add,
                    replica_groups=[list(range(NCORES))],
                    ins=[rs_in[:, :]],
                    outs=[rs_out[:, :]],
                )
                st["rs_out"] = rs_out

            def emit_tail(st):
                pooled_sb = wpool.tile([RPC, D], F32, tag="pooled_sb")
                nc.sync.dma_start(pooled_sb[:], st["rs_out"][:, :])
                _mlp_tail(nc, tc, cpool, ps_mlp, pooled_sb, identity,
                          w1t_sb, b1_sb, w2t_sb, b2a_sb, b2b_sb,
                          w3a_sb, w3b_sb, b3_sb, out)

            sts = []
            for rep in range(reps):
                sts.append(emit_front(rep))
                if rep >= 1:
                    emit_mid(sts[rep - 1])
                if rep >= 2:
                    emit_tail(sts[rep - 2])
            emit_mid(sts[-1])
            if reps >= 2:
                emit_tail(sts[-2])
            emit_tail(sts[-1])

    nc.compile()
    return nc


def make_in_maps(x, emb, idf, W1, b1, W2, b2, W3, b3):
    bf16 = mybir.dt.np(BF16)
    f8 = mybir.dt.np(F8)

    xt = np.asarray(x, dtype=np.int64).T  # [B, S]
    # token layout per core: tile t = r*STILES + f holds tokens s = p*16 + f
    xt4 = xt.reshape(B, P, STILES)  # [b, p, f]

    idf_pad = np.zeros(VPAD, dtype=np.float32)
    idf_pad[:V] = np.asarray(idf, dtype=np.float32)
    idf_pad[0] = 0.0  # pad token contributes nothing

    # fold idf into the embedding table: pooled = sum_v H_v^2 (idf_v emb_v)
    emb_pad = np.zeros((VPAD, D), dtype=np.float32)
    emb_pad[:V] = np.asarray(emb, dtype=np.float32)
    emb_pad *= idf_pad[:, None]
    emb_bf16 = emb_pad.astype(bf16)

    w1t = np.ascontiguousarray(np.asarray(W1, dtype=np.float32).T)
    w2t = np.ascontiguousarray(np.asarray(W2, dtype=np.float32).T)
    w3t = np.ascontiguousarray(np.asarray(W3, dtype=np.float32).T)
    b1 = np.ascontiguousarray(np.asarray(b1, dtype=np.float32))
    b2 = np.ascontiguousarray(np.asarray(b2, dtype=np.float32))
    b3 = np.ascontiguousarray(np.asarray(b3, dtype=np.float32))

    pp = np.arange(P)[:, None, None]  # [P, 1, 1]
    rr = np.arange(RPC)[None, :, None]
    ff = np.arange(STILES)[None, None, :]

    in_maps = []
    for c in range(NCORES):
        rows = xt4[c * RPC : (c + 1) * RPC]  # [r=8, p=128, f=16]
        tok = rows.transpose(1, 0, 2)  # [p, r, f]
        lo = (tok & (NLO - 1)).astype(np.int64)
        hi = (tok >> 7).astype(np.int64)
        ohlo = np.zeros((P, RPC, STILES, NLO), dtype=f8)
        ohhi = np.zeros((P, RPC, STILES, NHI), dtype=f8)
        one = f8(1.0)
        ohlo[pp, rr, ff, lo] = one
        ohhi[pp, rr, ff, hi] = one
        m = {
            "ohlo": ohlo,
            "ohhi": ohhi,
            "embs": np.ascontiguousarray(emb_bf16[c * VSH : (c + 1) * VSH]),
            "w1t": w1t, "b1": b1, "w2t": w2t, "b2": b2,
            "w3t": w3t, "b3": b3,
        }
        in_maps.append(m)
    return in_maps


def _get_nc(reps=1):
    key = f"nc5_r{reps}"
    if key not in _CACHE:
        _CACHE[key] = _build_nc(reps)
    return _CACHE[key]


class _Runner:
    """Cached jitted shard_map over the NEFF custom call (mirrors
    bass2jax.run_bass_via_pjrt, but reusable with device-resident inputs)."""

    def __init__(self, nc):
        import jax
        from jax.experimental.shard_map import shard_map
        from jax.sharding import Mesh, NamedSharding, PartitionSpec

        from concourse import bass2jax

        bass2jax.install_neuronx_cc_hook()
        assert nc.dbg_addr is None
        partition_name = (
            nc.partition_id_tensor.name if nc.partition_id_tensor else None
        )
        self._nc = nc
        self._partition_name = partition_name

        self.jax = jax
        in_names, out_names, out_avals, zero_outs = [], [], [], []
        for alloc in nc.m.functions[0].allocations:
            if not isinstance(alloc, mybir.MemoryLocationSet):
                continue
            name = alloc.memorylocations[0].name
            if alloc.kind == "ExternalInput":
                if name == partition_name:
                    continue
                in_names.append(name)
            elif alloc.kind == "ExternalOutput":
                out_names.append(name)
                shape = tuple(alloc.tensor_shape)
                dtype = mybir.dt.np(alloc.dtype)
                out_avals.append(jax.core.ShapedArray(shape, dtype))
                zero_outs.append(np.zeros((NCORES * shape[0], *shape[1:]), dtype))
        self.in_names = list(in_names)
        self.out_names = out_names
        self.out_avals = out_avals
        self.zero_outs = zero_outs
        n_params = len(in_names)
        n_outs = len(out_names)
        bind_names = tuple(
            in_names + out_names + ([partition_name] if partition_name else [])
        )
        donate = tuple(range(n_params, n_params + n_outs))

        def _body(*args):
            operands = list(args)
            if partition_name is not None:
                operands.append(bass2jax.partition_id_tensor())
            outs = bass2jax._bass_exec_p.bind(
                *operands,
                out_avals=tuple(out_avals),
                in_names=bind_names,
                out_names=tuple(out_names),
                lowering_input_output_aliases=(),
                sim_require_finite=True,
                sim_require_nnan=True,
                nc=nc,
            )
            return tuple(outs)

        devices = jax.devices()[:NCORES]
        self.mesh = Mesh(np.asarray(devices), ("core",))
        self.sharding = NamedSharding(self.mesh, PartitionSpec("core"))
        in_specs = (PartitionSpec("core"),) * (n_params + n_outs)
        out_specs = (PartitionSpec("core"),) * n_outs
        self.fn = jax.jit(
            shard_map(
                _body,
                mesh=self.mesh,
                in_specs=in_specs,
                out_specs=out_specs,
                check_rep=False,
            ),
            donate_argnums=donate,
            keep_unused=True,
        )

    def put_inputs(self, in_maps):
        concat = [
            np.concatenate([np.asarray(m[name]) for m in in_maps], axis=0)
            for name in self.in_names
        ]
        return [self.jax.device_put(a, self.sharding) for a in concat]

    def run(self, dev_in):
        zo = [self.jax.device_put(z, self.sharding) for z in self.zero_outs]
        outs = self.fn(*dev_in, *zo)
        self.jax.block_until_ready(outs)
        return outs

    def run_np(self, dev_in):
        outs = self.run(dev_in)
        return {
            name: np.asarray(outs[i]).reshape(NCORES, *self.out_avals[i].shape)
            for i, name in enumerate(self.out_names)
        }


def _get_runner(reps=1):
    key = f"runner5_r{reps}"
    if key not in _CACHE:
        _CACHE[key] = _Runner(_get_nc(reps))
    return _CACHE[key]


def kernel(x, emb, idf, W1, b1, W2, b2, W3, b3):
    in_maps = make_in_maps(x, emb, idf, W1, b1, W2, b2, W3, b3)
    runner = _get_runner(1)
    dev_in = runner.put_inputs(in_maps)
    outs = runner.run_np(dev_in)
    outp = np.concatenate([outs["out"][c] for c in range(NCORES)], axis=0)
    return outp.astype(np.float32)


# revision 12
# speedup vs baseline: 113.5531x; 1.5695x over previous
"""Trainium2 Bass kernel for tf-idf embedding pooling + MLP (v3).

Math identity: pooled[b] = sum_v c_{b,v}^2 * idf_v * emb_v where c = per-row
token counts. v = hi*128 + lo (radix 128 x 392). Per-core histogram of its 8
batch rows via one-hot matmuls H[lo, hi] += onehot_lo^T @ onehot_hi (one-hots
precomputed on host in fp8 — on-device is_equal generation measured 10-30x
slower than nominal DVE rate). AllToAll redistributes raw counts H by vocab
shard (hi ranges) in fp8 (counts <= 8 are exact in e4m3), receiving core
squares them fused with the reshuffle, then contracts its 6272-row bf16
idf-prescaled emb shard (emb*idf folded on host) for all 64 rows;
ReduceScatter returns final pooled rows; tiny MLP + softmax per core.

The rep loop is software-pipelined 3 deep (front: DMAs+hist+a2a | mid:
square+pooled+RS | tail: MLP+out) so rep k+1's histogram matmuls overlap rep
k's collectives — this also keeps the PE HAM-warm (the v2 profile showed the
whole histogram running at the cold 1.2 GHz clock because the ~30us
collective gap re-throttled the PE every rep).
"""

import sys

import numpy as np

sys.path.insert(0, "/opt/trn_rl_repo")

import concourse.bass as bass  # noqa: E402,F401
import concourse.mybir as mybir  # noqa: E402
import concourse.tile as tile  # noqa: E402
from concourse import bacc  # noqa: E402
from concourse.masks import make_identity  # noqa: E402

P = 128
S = 2048
B = 64
D = 256
V = 50000
NCORES = 8
RPC = B // NCORES  # 8 rows per core
NLO = 128
NHI = 392  # 391 real hi values + 1 pad (tokens never land there)
NHL = NHI // NCORES  # 49 hi rows per vocab shard
VSH = NHL * NLO  # 6272 vocab rows per shard
VPAD = NHI * NLO  # 50176
# tokens are bucketed on host into NG hi-groups of width W; each (row, group)
# gets GT tiles of 128 slots (empty slots carry all-zero one-hots). This cuts
# the hi one-hot DMA 4x (width W instead of NHI per token).
NG = 4
W = NHI // NG  # 98 hi bins per group
GT = 5  # tiles per (row, group): 640 slots for 512 +- ~20 tokens
TPR = NG * GT  # 20 token tiles per row

F32 = mybir.dt.float32
BF16 = mybir.dt.bfloat16
F8 = mybir.dt.float8e4

_CACHE = {}


def _mlp_tail(nc, tc, cpool, ps_mlp, pooled_sb, identity,
              w1t_sb, b1_sb, w2t_sb, b2a_sb, b2b_sb, w3a_sb, w3b_sb, b3_sb, out):
    """pooled_sb [RPC, 256] f32 -> softmax out DMA.

    All PSUM intermediates are disjoint column slices of one 512-col bank
    (PSUM pool allocation is bank-granular; 6 separate tags would need 6
    banks)."""
    mlp_ps = ps_mlp.tile([P, 64], F32, tag="mlp")
    pooledT = cpool.tile([P, 2, RPC], F32, tag="pooledT", bufs=2)
    for kc in range(2):
        ptp = mlp_ps[:, kc * RPC : (kc + 1) * RPC]
        nc.tensor.transpose(
            ptp, pooled_sb[:, kc * P : (kc + 1) * P], identity[:RPC, :RPC]
        )
        nc.vector.tensor_copy(pooledT[:, kc, :], ptp)

    h1_ps = mlp_ps[:100, 16:24]
    for kc in range(2):
        nc.tensor.matmul(
            h1_ps, lhsT=w1t_sb[:, kc, :], rhs=pooledT[:, kc, :],
            start=(kc == 0), stop=(kc == 1),
        )
    h1_sb = cpool.tile([100, RPC], F32, tag="h1_sb", bufs=2)
    nc.scalar.activation(
        h1_sb[:], h1_ps, mybir.ActivationFunctionType.Relu,
        bias=b1_sb[:, 0:1], scale=1.0,
    )

    h2a_ps = mlp_ps[:, 24:32]
    nc.tensor.matmul(h2a_ps, lhsT=w2t_sb[:, 0:128], rhs=h1_sb[:, :],
                     start=True, stop=True)
    h2b_ps = mlp_ps[:22, 32:40]
    nc.tensor.matmul(h2b_ps, lhsT=w2t_sb[:, 128:150], rhs=h1_sb[:, :],
                     start=True, stop=True)
    h2a_sb = cpool.tile([P, RPC], F32, tag="h2a_sb", bufs=2)
    h2b_sb = cpool.tile([22, RPC], F32, tag="h2b_sb", bufs=2)
    nc.scalar.activation(h2a_sb[:], h2a_ps,
                         mybir.ActivationFunctionType.Relu,
                         bias=b2a_sb[:, 0:1], scale=1.0)
    nc.scalar.activation(h2b_sb[:], h2b_ps,
                         mybir.ActivationFunctionType.Relu,
                         bias=b2b_sb[:, 0:1], scale=1.0)

    lg_ps = mlp_ps[:2, 40:48]
    nc.tensor.matmul(lg_ps, lhsT=w3a_sb[:, :], rhs=h2a_sb[:, :],
                     start=True, stop=False)
    nc.tensor.matmul(lg_ps, lhsT=w3b_sb[:, :], rhs=h2b_sb[:, :],
                     start=False, stop=True)
    lg_sb = cpool.tile([2, RPC], F32, tag="lg_sb", bufs=2)
    nc.scalar.add(lg_sb[:], lg_ps, b3_sb[:, 0:1])

    lt_ps = mlp_ps[:RPC, 48:50]
    nc.tensor.transpose(lt_ps, lg_sb[:, :], identity[:2, :2])
    e_sb = cpool.tile([RPC, 2], F32, tag="e_sb", bufs=2)
    nc.scalar.activation(e_sb[:], lt_ps[:, :], mybir.ActivationFunctionType.Exp)
    ssum = cpool.tile([RPC, 1], F32, tag="ssum", bufs=2)
    nc.vector.tensor_reduce(ssum[:], e_sb[:], axis=mybir.AxisListType.X,
                            op=mybir.AluOpType.add)
    rinv = cpool.tile([RPC, 1], F32, tag="rinv", bufs=2)
    nc.vector.reciprocal(rinv[:], ssum[:])
    res_sb = cpool.tile([RPC, 2], F32, tag="res_sb", bufs=2)
    nc.vector.tensor_scalar(out=res_sb[:], in0=e_sb[:], scalar1=rinv[:, 0:1],
                            scalar2=None, op0=mybir.AluOpType.mult)
    nc.sync.dma_start(out[:, :], res_sb[:])


def _build_nc(reps=1):
    nc = bacc.Bacc(None, target_bir_lowering=False, debug=False)

    ohlo = nc.dram_tensor("ohlo", [P, RPC, TPR, NLO], F8, kind="ExternalInput")
    ohhi = nc.dram_tensor("ohhi", [P, RPC, TPR, W], F8, kind="ExternalInput")
    embs = nc.dram_tensor("embs", [VSH, D], BF16, kind="ExternalInput")
    w1t = nc.dram_tensor("w1t", [256, 100], F32, kind="ExternalInput")
    b1 = nc.dram_tensor("b1", [100], F32, kind="ExternalInput")
    w2t = nc.dram_tensor("w2t", [100, 150], F32, kind="ExternalInput")
    b2 = nc.dram_tensor("b2", [150], F32, kind="ExternalInput")
    w3t = nc.dram_tensor("w3t", [150, 2], F32, kind="ExternalInput")
    b3 = nc.dram_tensor("b3", [2], F32, kind="ExternalInput")
    out = nc.dram_tensor("out", [RPC, 2], F32, kind="ExternalOutput")

    with tile.TileContext(nc) as tc:
        with (
            tc.tile_pool(name="const", bufs=1) as cpool,
            tc.tile_pool(name="oh_lo", bufs=2) as lopool,
            tc.tile_pool(name="oh_hi", bufs=2) as hipool,
            tc.tile_pool(name="emb", bufs=1) as epool,
            tc.tile_pool(name="work", bufs=2) as wpool,
            tc.tile_pool(name="sq", bufs=4) as sqpool,
            tc.tile_pool(name="dram", bufs=2, space="DRAM") as dpool,
            tc.tile_pool(name="ps_ht", bufs=2, space="PSUM") as ps_ht,
            tc.tile_pool(name="ps_acc", bufs=2, space="PSUM") as ps_acc,
            tc.tile_pool(name="ps_mlp", bufs=1, space="PSUM") as ps_mlp,
        ):
            # ---------- constants (amortized across reps, same as v1) ----------
            identity = cpool.tile([P, P], F32)
            make_identity(nc, identity[:])

            w1t_sb = cpool.tile([P, 2, 100], F32)
            nc.sync.dma_start(w1t_sb[:, :, :],
                              w1t[:, :].rearrange("(c p) m -> p c m", p=P))
            b1_sb = cpool.tile([100, 1], F32)
            nc.sync.dma_start(b1_sb[:, :], b1[:, None])
            w2t_sb = cpool.tile([100, 150], F32)
            nc.sync.dma_start(w2t_sb[:, :], w2t[:, :])
            b2a_sb = cpool.tile([128, 1], F32)
            b2b_sb = cpool.tile([22, 1], F32)
            nc.sync.dma_start(b2a_sb[:, :], b2[:128, None])
            nc.sync.dma_start(b2b_sb[:, :], b2[128:150, None])
            w3a_sb = cpool.tile([128, 2], F32)
            w3b_sb = cpool.tile([22, 2], F32)
            nc.sync.dma_start(w3a_sb[:, :], w3t[0:128, :])
            nc.sync.dma_start(w3b_sb[:, :], w3t[128:150, :])
            b3_sb = cpool.tile([2, 1], F32)
            nc.sync.dma_start(b3_sb[:, :], b3[:, None])

            def emit_front_hist(rep):
                """DMAs + per-row histograms + a2a input staging."""
                lo_sb, hi_sb = [], []
                for r in range(RPC):
                    lt = lopool.tile([P, TPR, NLO], F8, tag=f"lo{r}")
                    ht = hipool.tile([P, TPR, W], F8, tag=f"hi{r}")
                    nc.sync.dma_start(lt[:, :, :], ohlo[:, r, :, :])
                    nc.sync.dma_start(ht[:, :, :], ohhi[:, r, :, :])
                    lo_sb.append(lt)
                    hi_sb.append(ht)

                emb_sb = epool.tile([P, VSH // P, D], BF16, tag="emb_sb")
                nc.sync.dma_start(
                    emb_sb[:, :, :],
                    embs[:, :].rearrange("(c p) d -> p c d", p=P),
                )

                # a_sb layout [p, hi, r] bf16 raw counts: per dst_shard the
                # (hl, r) block is 49*8=392 contiguous elems for the a2a.
                a_sb = wpool.tile([P, NHI, RPC], BF16, tag="a_sb")
                for r in range(RPC):
                    ht_ps = ps_ht.tile([P, NHI], F32, tag="ht")
                    for g in range(NG):
                        for f in range(GT):
                            t = g * GT + f
                            nc.tensor.matmul(
                                ht_ps[:, g * W : (g + 1) * W],
                                lhsT=lo_sb[r][:, t, :],
                                rhs=hi_sb[r][:, t, :],
                                start=(f == 0), stop=(f == GT - 1))
                    if r % 2:
                        nc.vector.tensor_copy(a_sb[:, :, r], ht_ps[:, :])
                    else:
                        nc.scalar.copy(a_sb[:, :, r], ht_ps[:, :])

                a2a_in = dpool.tile([NCORES, P, NHL, RPC], BF16, tag="a2a_in")
                a2a_out = dpool.tile([NCORES, P, NHL, RPC], BF16, tag="a2a_out")
                nc.sync.dma_start(
                    a2a_in[:, :, :, :].rearrange("dst p hl r -> p dst (hl r)"),
                    a_sb[:, :, :].rearrange("p (dst hl) r -> p dst (hl r)",
                                            dst=NCORES),
                )
                return {"a2a_in": a2a_in, "a2a_out": a2a_out, "emb_sb": emb_sb}

            def emit_front_a2a(st):
                # emitted AFTER the previous rep's ReduceScatter so the
                # collective stream stays in natural (serial) order, while
                # this rep's histogram matmuls still precede the previous
                # rep's pooled matmuls in the PE stream.
                nc.gpsimd.collective_compute(
                    "AllToAll", mybir.AluOpType.bypass,
                    replica_groups=[list(range(NCORES))],
                    ins=[st["a2a_in"][:, :, :, :]],
                    outs=[st["a2a_out"][:, :, :, :]],
                )

            def emit_mid(st):
                """recv + fused square/reshuffle + pooled matmul + RS."""
                recv_sb = wpool.tile([P, NCORES, NHL, RPC], BF16, tag="recv_sb")
                nc.sync.dma_start(
                    recv_sb[:, :, :, :].rearrange("p src hl r -> p src (hl r)"),
                    st["a2a_out"][:, :, :, :].rearrange(
                        "src p hl r -> p src (hl r)"),
                )
                # square the counts fused with the [p][hl][(src r)] reshuffle
                recv_mm = wpool.tile([P, NHL, NCORES, RPC], BF16, tag="recv_mm")
                HLH = NHL // 2 + 1  # 25 | 24 split across ACT / DVE
                nc.scalar.activation(
                    recv_mm[:, :HLH, :, :],
                    recv_sb[:, :, :HLH, :].rearrange("p src hl r -> p hl src r"),
                    mybir.ActivationFunctionType.Square, scale=1.0)
                dve_in = recv_sb[:, :, HLH:, :].rearrange(
                    "p src hl r -> p hl src r")
                nc.vector.tensor_tensor(
                    out=recv_mm[:, HLH:, :, :], in0=dve_in, in1=dve_in,
                    op=mybir.AluOpType.mult)

                pooled_ps = ps_acc.tile([B, D], F32, tag="pooled")
                emb_sb = st["emb_sb"]
                for c in range(VSH // P):  # 49 chunks of 128 vocab rows
                    nc.tensor.matmul(
                        pooled_ps[:, :],
                        lhsT=recv_mm[:, c, :, :].rearrange(
                            "p src r -> p (src r)"),
                        rhs=emb_sb[:, c, :],
                        start=(c == 0), stop=(c == VSH // P - 1))
                pooled_full = wpool.tile([B, D], F32, tag="pooled_full")
                nc.vector.tensor_copy(pooled_full[:], pooled_ps[:, :])

                rs_in = dpool.tile([B, D], F32, tag="rs_in")
                rs_out = dpool.tile([RPC, D], F32, tag="rs_out")
                nc.sync.dma_start(rs_in[:, :], pooled_full[:])
                nc.gpsimd.collective_compute(
                    "ReduceScatter", mybir.AluOpType.add,
                    replica_groups=[list(range(NCORES))],
                    ins=[rs_in[:, :]],
                    outs=[rs_out[:, :]],
                )
                st["rs_out"] = rs_out

            def emit_tail(st):
                pooled_sb = wpool.tile([RPC, D], F32, tag="pooled_sb")
                nc.sync.dma_start(pooled_sb[:], st["rs_out"][:, :])
                _mlp_tail(nc, tc, cpool, ps_mlp, pooled_sb, identity,
                          w1t_sb, b1_sb, w2t_sb, b2a_sb, b2b_sb,
                          w3a_sb, w3b_sb, b3_sb, out)

            sts = []
            for rep in range(reps):
                sts.append(emit_front_hist(rep))
                if rep >= 1:
                    emit_mid(sts[rep - 1])
                emit_front_a2a(sts[rep])
                if rep >= 2:
                    emit_tail(sts[rep - 2])
            emit_mid(sts[-1])
            if reps >= 2:
                emit_tail(sts[-2])
            emit_tail(sts[-1])

    nc.compile()
    return nc


def make_in_maps(x, emb, idf, W1, b1, W2, b2, W3, b3):
    bf16 = mybir.dt.np(BF16)
    f8 = mybir.dt.np(F8)

    xt = np.asarray(x, dtype=np.int64).T  # [B, S]

    idf_pad = np.zeros(VPAD, dtype=np.float32)
    idf_pad[:V] = np.asarray(idf, dtype=np.float32)
    idf_pad[0] = 0.0  # pad token contributes nothing

    # fold idf into the embedding table: pooled = sum_v H_v^2 (idf_v emb_v)
    emb_pad = np.zeros((VPAD, D), dtype=np.float32)
    emb_pad[:V] = np.asarray(emb, dtype=np.float32)
    emb_pad *= idf_pad[:, None]
    emb_bf16 = emb_pad.astype(bf16)

    w1t = np.ascontiguousarray(np.asarray(W1, dtype=np.float32).T)
    w2t = np.ascontiguousarray(np.asarray(W2, dtype=np.float32).T)
    w3t = np.ascontiguousarray(np.asarray(W3, dtype=np.float32).T)
    b1 = np.ascontiguousarray(np.asarray(b1, dtype=np.float32))
    b2 = np.ascontiguousarray(np.asarray(b2, dtype=np.float32))
    b3 = np.ascontiguousarray(np.asarray(b3, dtype=np.float32))

    in_maps = []
    one = f8(1.0)
    for c in range(NCORES):
        rows = xt[c * RPC : (c + 1) * RPC]  # [r=8, s=2048]
        ohlo = np.zeros((P, RPC, TPR, NLO), dtype=f8)
        ohhi = np.zeros((P, RPC, TPR, W), dtype=f8)
        for r in range(RPC):
            tok = rows[r]
            lo = (tok & (NLO - 1)).astype(np.int64)
            hi = (tok >> 7).astype(np.int64)
            g = hi // W
            order = np.argsort(g, kind="stable")
            gs = g[order]
            starts = np.searchsorted(gs, gs)  # group start index per token
            k = np.arange(S) - starts  # rank within group
            assert k.max() < GT * P, "hi-group overflow: raise GT"
            p = k % P
            f = k // P
            t = gs * GT + f
            ohlo[p, r, t, lo[order]] = one
            ohhi[p, r, t, hi[order] - gs * W] = one
        m = {
            "ohlo": ohlo,
            "ohhi": ohhi,
            "embs": np.ascontiguousarray(emb_bf16[c * VSH : (c + 1) * VSH]),
            "w1t": w1t, "b1": b1, "w2t": w2t, "b2": b2,
            "w3t": w3t, "b3": b3,
        }
        in_maps.append(m)
    return in_maps


def _get_nc(reps=1):
    key = f"nc5_r{reps}"
    if key not in _CACHE:
        _CACHE[key] = _build_nc(reps)
    return _CACHE[key]


class _Runner:
    """Cached jitted shard_map over the NEFF custom call (mirrors
    bass2jax.run_bass_via_pjrt, but reusable with device-resident inputs)."""

    def __init__(self, nc):
        import jax
        from jax.experimental.shard_map import shard_map
        from jax.sharding import Mesh, NamedSharding, PartitionSpec

        from concourse import bass2jax

        bass2jax.install_neuronx_cc_hook()
        assert nc.dbg_addr is None
        partition_name = (
            nc.partition_id_tensor.name if nc.partition_id_tensor else None
        )
        self._nc = nc
        self._partition_name = partition_name

        self.jax = jax
        in_names, out_names, out_avals, zero_outs = [], [], [], []
        for alloc in nc.m.functions[0].allocations:
            if not isinstance(alloc, mybir.MemoryLocationSet):
                continue
            name = alloc.memorylocations[0].name
            if alloc.kind == "ExternalInput":
                if name == partition_name:
                    continue
                in_names.append(name)
            elif alloc.kind == "ExternalOutput":
                out_names.append(name)
                shape = tuple(alloc.tensor_shape)
                dtype = mybir.dt.np(alloc.dtype)
                out_avals.append(jax.core.ShapedArray(shape, dtype))
                zero_outs.append(np.zeros((NCORES * shape[0], *shape[1:]), dtype))
        self.in_names = list(in_names)
        self.out_names = out_names
        self.out_avals = out_avals
        self.zero_outs = zero_outs
        n_params = len(in_names)
        n_outs = len(out_names)
        bind_names = tuple(
            in_names + out_names + ([partition_name] if partition_name else [])
        )
        donate = tuple(range(n_params, n_params + n_outs))

        def _body(*args):
            operands = list(args)
            if partition_name is not None:
                operands.append(bass2jax.partition_id_tensor())
            outs = bass2jax._bass_exec_p.bind(
                *operands,
                out_avals=tuple(out_avals),
                in_names=bind_names,
                out_names=tuple(out_names),
                lowering_input_output_aliases=(),
                sim_require_finite=True,
                sim_require_nnan=True,
                nc=nc,
            )
            return tuple(outs)

        devices = jax.devices()[:NCORES]
        self.mesh = Mesh(np.asarray(devices), ("core",))
        self.sharding = NamedSharding(self.mesh, PartitionSpec("core"))
        in_specs = (PartitionSpec("core"),) * (n_params + n_outs)
        out_specs = (PartitionSpec("core"),) * n_outs
        self.fn = jax.jit(
            shard_map(
                _body,
                mesh=self.mesh,
                in_specs=in_specs,
                out_specs=out_specs,
                check_rep=False,
            ),
            donate_argnums=donate,
            keep_unused=True,
        )

    def put_inputs(self, in_maps):
        concat = [
            np.concatenate([np.asarray(m[name]) for m in in_maps], axis=0)
            for name in self.in_names
        ]
        return [self.jax.device_put(a, self.sharding) for a in concat]

    def run(self, dev_in):
        zo = [self.jax.device_put(z, self.sharding) for z in self.zero_outs]
        outs = self.fn(*dev_in, *zo)
        self.jax.block_until_ready(outs)
        return outs

    def run_np(self, dev_in):
        outs = self.run(dev_in)
        return {
            name: np.asarray(outs[i]).reshape(NCORES, *self.out_avals[i].shape)
            for i, name in enumerate(self.out_names)
        }


def _get_runner(reps=1):
    key = f"runner5_r{reps}"
    if key not in _CACHE:
        _CACHE[key] = _Runner(_get_nc(reps))
    return _CACHE[key]


def kernel(x, emb, idf, W1, b1, W2, b2, W3, b3):
    in_maps = make_in_maps(x, emb, idf, W1, b1, W2, b2, W3, b3)
    runner = _get_runner(1)
    dev_in = runner.put_inputs(in_maps)
    outs = runner.run_np(dev_in)
    outp = np.concatenate([outs["out"][c] for c in range(NCORES)], axis=0)
    return outp.astype(np.float32)
